# revision 3
# baseline (speedup 1.0000x reference)
import numpy as np

# nn_NearestNeighbours: batch [8,512,512] f32, emb [50000,512] f32,
# output argmin cosine-distance indices [8,512] int32.
#
# Strategy: vocab-sharded fp8 DoubleRow screen GEMM over 8 cores
# (6144 vocab cols per core, 49152 total; the 848-col tail is scored
# exactly on the host). Both sides are L2-normalized and scaled by
# sqrt(E) on the host before fp8 quantization, so screen scores are
# ~512*cosine and embedding-norm variance never eats into the margin.
#
# Per 128-row m-tile the GEMM runs as 3 pairs of 1024-col PSUM chunks;
# within a pair the stationary (batch block) is held for runs of 4
# matmuls (p outer) to keep the PE at its ~263ns/instr cadence.
# Eviction per tile: ACT copies chunks 0,1,2,4 to f16; DVE pair-maxes
# chunk3 vs a0 and chunk5 vs a1 straight out of PSUM (retiring PSUM at
# half the DVE cost of a copy), then two f16 folds produce a 2048-wide
# cell table: cells [0,1024) = max of 4 source cols, [1024,2048) = max
# of 2. The host expands cells within MARGIN of the row-global best and
# exact-rescores candidates in f64 (plus the tail block).
B, S, E, V = 8, 512, 512, 50000
R = B * S              # 4096 token rows
NC = 8                 # cores
VS = 6144              # vocab cols per core on device
VTAIL = NC * VS        # 49152; [VTAIL, V) scored on host
KT = E // 128          # 4 k-subtiles
MT = R // 128          # 32 m-tiles
CHW = 1024             # psum chunk width (6 chunks per m-tile)
FW = 2048              # folded cell-table width per m-tile
SCALE = 22.62741699796952  # sqrt(512); screen score ~ SCALE^2 * cosine
MARGIN = 16.0          # screen-score pruning margin

_CACHE = {}


def _build():
    import concourse.bacc as bacc
    import concourse.mybir as mybir
    from concourse.tile import TileContext

    dtf = mybir.dt.float32
    dt8 = mybir.dt.float8e4
    dth = mybir.dt.float16
    DR = mybir.MatmulPerfMode.DoubleRow
    Copy = mybir.ActivationFunctionType.Copy

    nc = bacc.Bacc("TRN2", target_bir_lowering=False, debug=False)
    bT_ap = nc.dram_tensor("bT", [E, R], dt8, kind="ExternalInput").ap()
    embT_ap = nc.dram_tensor("embT", [E, VS], dt8, kind="ExternalInput").ap()
    outF_ap = nc.dram_tensor("outF", [R, FW], dth, kind="ExternalOutput").ap()

    with TileContext(nc) as tc:
        with tc.sbuf_pool(name="emb", bufs=1) as embp, \
             tc.sbuf_pool(name="bt", bufs=2) as btp, \
             tc.sbuf_pool(name="wk", bufs=2) as wkp, \
             tc.psum_pool(name="ps", bufs=2) as ps:
            engs = [nc.sync, nc.scalar, nc.gpsimd]
            gt = btp.tile([128, KT, 512], dt8)
            for k in range(KT):
                engs[k % 3].dma_start(gt[:, k:k + 1, :],
                                      bT_ap[128 * k:128 * (k + 1), 0:512])
            emb8 = embp.tile([128, KT, VS], dt8, name="emb8")
            # critical pieces first (first matmuls need ktiles 0,1 low cols),
            # small at the front so the PE starts ASAP; rest 1024-wide
            order = []
            for k in (0, 1):
                order.append((k, 0, 256))
            for k in (0, 1):
                order.append((k, 256, 512))
            for k in (2, 3):
                order.append((k, 0, 256))
            for k in (2, 3):
                order.append((k, 256, 512))
            for k in range(KT):
                order.append((k, 512, 1024))
            for off in range(1024, VS, 1024):
                for k in range(KT):
                    order.append((k, off, off + 1024))
            for i, (k, lo, hi) in enumerate(order):
                engs[i % 3].dma_start(
                    emb8[:, k:k + 1, lo:hi],
                    embT_ap[128 * k:128 * (k + 1), lo:hi])

            for m in range(MT):
                mm = m % 4
                cur = gt
                if mm == 0 and (m // 4 + 1) < MT // 4:
                    g1 = m // 4 + 1
                    gt = btp.tile([128, KT, 512], dt8)
                    for k in range(KT):
                        nc.sync.dma_start(
                            gt[:, k:k + 1, :],
                            bT_ap[128 * k:128 * (k + 1), 512 * g1:512 * (g1 + 1)])
                a = wkp.tile([128, 4, CHW], dth, name="a")
                bb = wkp.tile([128, 2, CHW], dth, name="bb")
                ot = wkp.tile([128, FW], dth, name="ot")
                for g in range(3):
                    pt0 = ps.tile([128, CHW], dtf)
                    pt1 = ps.tile([128, CHW], dtf)
                    for p in range(2):
                        for ci, pt in ((0, pt0), (1, pt1)):
                            for k in range(2):
                                col = (2 * g + ci) * CHW + 512 * k
                                nc.tensor.matmul(
                                    pt[:, 512 * k:512 * (k + 1)],
                                    cur[:, 2 * p:2 * p + 2, 128 * mm:128 * mm + 128],
                                    emb8[:, 2 * p:2 * p + 2, col:col + 512],
                                    start=(p == 0), stop=(p == 1), perf_mode=DR)
                    if g == 0:
                        nc.scalar.activation(a[:, 0, :], pt0[:], Copy)   # ch0
                        nc.scalar.activation(a[:, 1, :], pt1[:], Copy)   # ch1
                    elif g == 1:
                        nc.vector.tensor_max(bb[:, 0, :], pt1[:], a[:, 0, :])  # ch3+ch0
                        nc.scalar.activation(a[:, 2, :], pt0[:], Copy)   # ch2
                    else:
                        nc.scalar.activation(a[:, 3, :], pt0[:], Copy)   # ch4
                        nc.vector.tensor_max(bb[:, 1, :], pt1[:], a[:, 1, :])  # ch5+ch1
                        nc.vector.tensor_max(ot[:, CHW:FW], a[:, 2, :], a[:, 3, :])
                        nc.vector.tensor_max(ot[:, 0:CHW], bb[:, 0, :], bb[:, 1, :])
                        nc.gpsimd.dma_start(
                            outF_ap[128 * m:128 * (m + 1), :], ot[:, :])
    nc.compile()
    return nc


def _run(batch: np.ndarray, emb: np.ndarray, trace: bool = False):
    import ml_dtypes
    from concourse import bass_utils

    if "nc" not in _CACHE:
        _CACHE["nc"] = _build()
    nc = _CACHE["nc"]
    f8 = ml_dtypes.float8_e4m3

    b = np.ascontiguousarray(batch.reshape(R, E)).astype(np.float64)
    binv = SCALE / np.sqrt((b * b).sum(axis=1) + 1e-30)
    bn = (b * binv[:, None]).astype(np.float32)
    bT8 = np.ascontiguousarray(bn.T).astype(f8)

    emb64 = emb.astype(np.float64)
    einv = 1.0 / np.sqrt((emb64 * emb64).sum(axis=1) + 1e-30)
    en = (emb64[:VTAIL] * (SCALE * einv[:VTAIL])[:, None]).astype(np.float32)
    embT8 = en.T.astype(f8)
    in_maps = []
    for c in range(NC):
        in_maps.append({
            "bT": bT8,
            "embT": np.ascontiguousarray(embT8[:, c * VS:(c + 1) * VS]),
        })

    res = bass_utils.run_bass_kernel_spmd(
        nc, in_maps, core_ids=list(range(NC)), trace=trace
    )

    # [R, NC, FW] screen cell table; cells [0,CHW) fold cols
    # {j, 1024+j, 3072+j, 5120+j}, cells [CHW,FW) fold {2048+j, 4096+j}
    Fall = np.stack(
        [np.asarray(res.results[c]["outF"]) for c in range(NC)], axis=1
    ).astype(np.float32)
    gbest = Fall.max(axis=(1, 2))
    rows, cores, cells = np.nonzero(Fall >= (gbest - MARGIN)[:, None, None])

    m4 = cells < CHW
    r4, c4, j4 = rows[m4], cores[m4], cells[m4]
    r2, c2, j2 = rows[~m4], cores[~m4], cells[~m4] - CHW
    cand4 = (c4 * VS)[:, None] + j4[:, None] + \
        np.array([0, 1024, 3072, 5120])[None, :]
    cand2 = (c2 * VS)[:, None] + j2[:, None] + \
        np.array([2048, 4096])[None, :]
    cand = np.concatenate([cand4.reshape(-1), cand2.reshape(-1)])
    crow = np.concatenate([np.repeat(r4, 4), np.repeat(r2, 2)])

    # exact rescore in f64 on normalized embeddings
    s = np.einsum("ij,ij->i",
                  emb64[cand] * einv[cand][:, None], b[crow])

    best_s = np.full(R, -np.inf)
    best_i = np.zeros(R, np.int64)
    np.maximum.at(best_s, crow, s)
    hit = s == best_s[crow]
    best_i[crow[hit]] = cand[hit]

    # exact tail block [VTAIL, V)
    tail = (emb64[VTAIL:] * einv[VTAIL:, None]) @ b.T   # [848, R]
    t_best = tail.max(axis=0)
    t_idx = VTAIL + tail.argmax(axis=0)
    use_tail = t_best > best_s
    best_i[use_tail] = t_idx[use_tail]

    return best_i.astype(np.int32).reshape(B, S), res


def kernel(batch: np.ndarray, emb: np.ndarray) -> np.ndarray:
    out, _ = _run(batch, emb, trace=False)
    return out


# revision 4
# speedup vs baseline: 1.1828x; 1.1828x over previous
import numpy as np

# nn_NearestNeighbours: batch [8,512,512] f32, emb [50000,512] f32,
# output argmin cosine-distance indices [8,512] int32.
#
# Strategy: vocab-sharded fp8 DoubleRow screen GEMM over 8 cores
# (6144 vocab cols per core, 49152 total; the 848-col tail is scored
# exactly on the host). Both sides are L2-normalized and scaled by
# sqrt(E) on the host before fp8 quantization, so screen scores are
# ~512*cosine and embedding-norm variance never eats into the margin.
#
# Per 128-row m-tile the GEMM runs as 3 pairs of 1024-col PSUM chunks;
# within a pair the stationary (batch block) is held for runs of 4
# matmuls (p outer) to keep the PE at its ~263ns/instr cadence.
# Eviction per tile: ACT copies chunks 0,1,2,4 to f16; DVE pair-maxes
# chunk3 vs a0 and chunk5 vs a1 straight out of PSUM (retiring PSUM at
# half the DVE cost of a copy), then two f16 folds produce a 2048-wide
# cell table: cells [0,1024) = max of 4 source cols, [1024,2048) = max
# of 2. The host expands cells within MARGIN of the row-global best and
# exact-rescores candidates in f64 (plus the tail block).
B, S, E, V = 8, 512, 512, 50000
R = B * S              # 4096 token rows
NC = 8                 # cores
VS = 6144              # vocab cols per core on device
VTAIL = NC * VS        # 49152; [VTAIL, V) scored on host
KT = E // 128          # 4 k-subtiles
MT = R // 128          # 32 m-tiles
CHW = 1024             # psum chunk width (6 chunks per m-tile)
FW = 2048              # folded cell-table width per m-tile
SCALE = 22.62741699796952  # sqrt(512); screen score ~ SCALE^2 * cosine
MARGIN = 16.0          # screen-score pruning margin

_CACHE = {}


def _build():
    import concourse.bacc as bacc
    import concourse.mybir as mybir
    from concourse.tile import TileContext

    dtf = mybir.dt.float32
    dt8 = mybir.dt.float8e4
    dth = mybir.dt.float16
    DR = mybir.MatmulPerfMode.DoubleRow
    Copy = mybir.ActivationFunctionType.Copy

    nc = bacc.Bacc("TRN2", target_bir_lowering=False, debug=False)
    bT_ap = nc.dram_tensor("bT", [E, R], dt8, kind="ExternalInput").ap()
    embT_ap = nc.dram_tensor("embT", [E, VS], dt8, kind="ExternalInput").ap()
    outF_ap = nc.dram_tensor("outF", [R, FW], dth, kind="ExternalOutput").ap()

    with TileContext(nc) as tc:
        with tc.sbuf_pool(name="emb", bufs=1) as embp, \
             tc.sbuf_pool(name="bt", bufs=2) as btp, \
             tc.sbuf_pool(name="wk", bufs=2) as wkp, \
             tc.psum_pool(name="ps", bufs=2) as ps:
            engs = [nc.sync, nc.scalar, nc.gpsimd]
            gt = btp.tile([128, KT, 512], dt8)
            for k in range(KT):
                engs[k % 3].dma_start(gt[:, k:k + 1, :],
                                      bT_ap[128 * k:128 * (k + 1), 0:512])
            emb8 = embp.tile([128, KT, VS], dt8, name="emb8")
            # critical pieces first (first matmuls need ktiles 0,1 low cols),
            # small at the front so the PE starts ASAP; rest 1024-wide
            order = []
            for k in (0, 1):
                order.append((k, 0, 256))
            for k in (0, 1):
                order.append((k, 256, 512))
            for k in (2, 3):
                order.append((k, 0, 256))
            for k in (2, 3):
                order.append((k, 256, 512))
            for k in range(KT):
                order.append((k, 512, 1024))
            for off in range(1024, VS, 1024):
                for k in range(KT):
                    order.append((k, off, off + 1024))
            for i, (k, lo, hi) in enumerate(order):
                engs[i % 3].dma_start(
                    emb8[:, k:k + 1, lo:hi],
                    embT_ap[128 * k:128 * (k + 1), lo:hi])

            cur = gt
            for m in range(MT):
                mm = m % 4
                if mm == 0:
                    cur = gt
                    if (m // 4 + 1) < MT // 4:
                        g1 = m // 4 + 1
                        gt = btp.tile([128, KT, 512], dt8)
                        for k in range(KT):
                            nc.sync.dma_start(
                                gt[:, k:k + 1, :],
                                bT_ap[128 * k:128 * (k + 1), 512 * g1:512 * (g1 + 1)])
                a = wkp.tile([128, 4, CHW], dth, name="a")
                bb = wkp.tile([128, 2, CHW], dth, name="bb")
                ot = wkp.tile([128, FW], dth, name="ot")
                for g in range(3):
                    pt0 = ps.tile([128, CHW], dtf)
                    pt1 = ps.tile([128, CHW], dtf)
                    for p in range(2):
                        for ci, pt in ((0, pt0), (1, pt1)):
                            for k in range(2):
                                col = (2 * g + ci) * CHW + 512 * k
                                nc.tensor.matmul(
                                    pt[:, 512 * k:512 * (k + 1)],
                                    cur[:, 2 * p:2 * p + 2, 128 * mm:128 * mm + 128],
                                    emb8[:, 2 * p:2 * p + 2, col:col + 512],
                                    start=(p == 0), stop=(p == 1), perf_mode=DR)
                    if g == 0:
                        nc.scalar.activation(a[:, 0, :], pt0[:], Copy)   # ch0
                        nc.scalar.activation(a[:, 1, :], pt1[:], Copy)   # ch1
                    elif g == 1:
                        nc.vector.tensor_max(bb[:, 0, :], pt1[:], a[:, 0, :])  # ch3+ch0
                        nc.scalar.activation(a[:, 2, :], pt0[:], Copy)   # ch2
                    else:
                        nc.scalar.activation(a[:, 3, :], pt0[:], Copy)   # ch4
                        nc.vector.tensor_max(bb[:, 1, :], pt1[:], a[:, 1, :])  # ch5+ch1
                        nc.vector.tensor_max(ot[:, CHW:FW], a[:, 2, :], a[:, 3, :])
                        nc.vector.tensor_max(ot[:, 0:CHW], bb[:, 0, :], bb[:, 1, :])
                        nc.gpsimd.dma_start(
                            outF_ap[128 * m:128 * (m + 1), :], ot[:, :])
    nc.compile()
    return nc


def _run(batch: np.ndarray, emb: np.ndarray, trace: bool = False):
    import ml_dtypes
    from concourse import bass_utils

    if "nc" not in _CACHE:
        _CACHE["nc"] = _build()
    nc = _CACHE["nc"]
    f8 = ml_dtypes.float8_e4m3

    b = np.ascontiguousarray(batch.reshape(R, E)).astype(np.float64)
    binv = SCALE / np.sqrt((b * b).sum(axis=1) + 1e-30)
    bn = (b * binv[:, None]).astype(np.float32)
    bT8 = np.ascontiguousarray(bn.T).astype(f8)

    emb64 = emb.astype(np.float64)
    einv = 1.0 / np.sqrt((emb64 * emb64).sum(axis=1) + 1e-30)
    en = (emb64[:VTAIL] * (SCALE * einv[:VTAIL])[:, None]).astype(np.float32)
    embT8 = en.T.astype(f8)
    in_maps = []
    for c in range(NC):
        in_maps.append({
            "bT": bT8,
            "embT": np.ascontiguousarray(embT8[:, c * VS:(c + 1) * VS]),
        })

    res = bass_utils.run_bass_kernel_spmd(
        nc, in_maps, core_ids=list(range(NC)), trace=trace
    )

    # [R, NC, FW] screen cell table; cells [0,CHW) fold cols
    # {j, 1024+j, 3072+j, 5120+j}, cells [CHW,FW) fold {2048+j, 4096+j}
    Fall = np.stack(
        [np.asarray(res.results[c]["outF"]) for c in range(NC)], axis=1
    ).astype(np.float32)
    gbest = Fall.max(axis=(1, 2))
    rows, cores, cells = np.nonzero(Fall >= (gbest - MARGIN)[:, None, None])

    m4 = cells < CHW
    r4, c4, j4 = rows[m4], cores[m4], cells[m4]
    r2, c2, j2 = rows[~m4], cores[~m4], cells[~m4] - CHW
    cand4 = (c4 * VS)[:, None] + j4[:, None] + \
        np.array([0, 1024, 3072, 5120])[None, :]
    cand2 = (c2 * VS)[:, None] + j2[:, None] + \
        np.array([2048, 4096])[None, :]
    cand = np.concatenate([cand4.reshape(-1), cand2.reshape(-1)])
    crow = np.concatenate([np.repeat(r4, 4), np.repeat(r2, 2)])

    # exact rescore in f64 on normalized embeddings
    s = np.einsum("ij,ij->i",
                  emb64[cand] * einv[cand][:, None], b[crow])

    best_s = np.full(R, -np.inf)
    best_i = np.zeros(R, np.int64)
    np.maximum.at(best_s, crow, s)
    hit = s == best_s[crow]
    best_i[crow[hit]] = cand[hit]

    # exact tail block [VTAIL, V)
    tail = (emb64[VTAIL:] * einv[VTAIL:, None]) @ b.T   # [848, R]
    t_best = tail.max(axis=0)
    t_idx = VTAIL + tail.argmax(axis=0)
    use_tail = t_best > best_s
    best_i[use_tail] = t_idx[use_tail]

    return best_i.astype(np.int32).reshape(B, S), res


def kernel(batch: np.ndarray, emb: np.ndarray) -> np.ndarray:
    out, _ = _run(batch, emb, trace=False)
    return out
